# revision 66
# baseline (speedup 1.0000x reference)
"""Trainium2 Bass kernel for CustomMultiHeadAttention with relative position
bias (Music-Transformer skew), causal mask.

Sharding: pure data-parallel over batch - B=8 batches, one per NeuronCore.
Weights and Er are replicated. No collectives.

Per-core pipeline (all matmuls f16 inputs, f32 PSUM accumulate):
  - Prologue: load Q/K/V/W natural f32, cast f16, XBAR DMA-transpose into
    contraction-major layouts (no PE transposes anywhere).
  - Projections: qT/kT [dout, s] f16, v16 [t, d] f16 natural.
  - Per head: QEr strips -> f16 DRAM scratch laid out with -65504-filled
    128-col gaps between causal-packed strips; diagonal-AP HWDGE read
    realizes the skew AND the causal mask (gap values -> exp ~ 0).
    Scores computed natural [s-strip, t]: QK matmul -> PSUM, vector adds
    srel, scalar Exp (scale=1/8) with accum_out row-sums -> Z; reciprocal
    on [128,1] columns; gpsimd normalizes A in [s,t] layout; XBAR
    DMA-transpose A -> A16T [t, s]; AV accumulates both heads of a pair
    into one [128, S] PSUM tile (odd head at partition base 64).
  - Output projection f16 + XBAR transpose + bias -> DMA out.
"""

import numpy as np

import concourse.bass as bass
import concourse.tile as tile
from concourse import bacc, mybir
from concourse.bass import AP
from concourse.bass_utils import run_bass_kernel_spmd

N_CORES = 8
B, S, D, H, DK = 8, 1024, 768, 12, 64
NT = S // 128          # 8 s-tiles
NI = D // 128          # 6 d-blocks
f32 = mybir.dt.float32
f16 = mybir.dt.float16
NEG16 = -65504.0       # f16 lowest; exp(0.125*(x+NEG16)) == 0

# causal-packed column offsets (strip si width 128*(si+1))
OFFS = [0]
for _si in range(NT):
    OFFS.append(OFFS[-1] + 128 * (_si + 1))
SREL_W = OFFS[-1]                    # 4608
# gapped DRAM scratch offsets: strip si at OFFSP[si], then 128-col gap
OFFSP = [0]
for _si in range(NT):
    OFFSP.append(OFFSP[-1] + 128 * (_si + 1) + 128)
RSW = OFFSP[-1]                      # 5632
N_REG = 12                           # DRAM scratch regions (one per head)


def build_nc():
    nc = bacc.Bacc("TRN2", target_bir_lowering=False, debug=False,
                   num_devices=N_CORES)

    Qb = nc.dram_tensor("Qb", [S, D], f32, kind="ExternalInput")
    Kb = nc.dram_tensor("Kb", [S, D], f32, kind="ExternalInput")
    Vb = nc.dram_tensor("Vb", [S, D], f32, kind="ExternalInput")
    Wq = nc.dram_tensor("Wq", [D, D], f32, kind="ExternalInput")
    Wk = nc.dram_tensor("Wk", [D, D], f32, kind="ExternalInput")
    Wv = nc.dram_tensor("Wv", [D, D], f32, kind="ExternalInput")
    Wo = nc.dram_tensor("Wo", [D, D], f32, kind="ExternalInput")
    bq = nc.dram_tensor("bq", [D], f32, kind="ExternalInput")
    bk = nc.dram_tensor("bk", [D], f32, kind="ExternalInput")
    bv = nc.dram_tensor("bv", [D], f32, kind="ExternalInput")
    bo = nc.dram_tensor("bo", [D], f32, kind="ExternalInput")
    Er = nc.dram_tensor("Er", [S, DK], f32, kind="ExternalInput")
    out = nc.dram_tensor("out", [S, D], f32, kind="ExternalOutput")
    qer_dram = nc.dram_tensor("qer_scratch", [N_REG * 128 * RSW], f16)

    import os
    if os.environ.get("KDBG"):
        dbg = {
            "d_qT0": nc.dram_tensor("d_qT0", [128, S], f32, kind="ExternalOutput"),
            "d_kT0": nc.dram_tensor("d_kT0", [128, S], f32, kind="ExternalOutput"),
            "d_v0": nc.dram_tensor("d_v0", [128, D], f32, kind="ExternalOutput"),
            "d_er": nc.dram_tensor("d_er", [128, S], f32, kind="ExternalOutput"),
            "d_s16": nc.dram_tensor("d_s16", [128, SREL_W], f32, kind="ExternalOutput"),
            "d_a16": nc.dram_tensor("d_a16", [128, SREL_W], f32, kind="ExternalOutput"),
            "d_a16T": nc.dram_tensor("d_a16T", [128, NT * S], f32, kind="ExternalOutput"),
            "d_ao0": nc.dram_tensor("d_ao0", [128, S], f32, kind="ExternalOutput"),
            "d_z": nc.dram_tensor("d_z", [128, 16], f32, kind="ExternalOutput"),
        }
    else:
        dbg = None

    tensors = dict(Qb=Qb, Kb=Kb, Vb=Vb, Wq=Wq, Wk=Wk, Wv=Wv, Wo=Wo,
                   bq=bq, bk=bk, bv=bv, bo=bo, Er=Er, out=out,
                   qer_dram=qer_dram, dbg=dbg)
    with tile.TileContext(nc) as tc:
        _build_body(nc, tc, tensors)
    nc.compile()
    return nc


def _pe_transpose_in(nc, stg, helpers, src_dram, nrow, ncol, dst, blk_stride):
    """Load [nrow, ncol] f32 DRAM natural (one DMA per half-tensor),
    PE-transpose 128x128 blocks (f32), evict f32->f16 with a strided AP
    into dst [128, (ncol//128)*blk_stride]: src-col-block ct lands at
    dst cols [ct*blk_stride + rt*128, ...) for src-row-block rt."""
    nrb = nrow // 128
    nb = nrb // 2              # row-blocks per load (2 loads per tensor)
    ident16, ps = helpers["ident16"], helpers["ps"]
    ev = helpers["ev"]
    for ld in range(2):
        r0 = ld * nb
        stage = stg.tile([128, nb * ncol], f32, tag="nat_stage")
        src = AP(tensor=src_dram, offset=r0 * 128 * ncol,
                 ap=[[ncol, 128], [128 * ncol, nb], [1, ncol]])
        nc.sync.dma_start(out=stage[:], in_=src)
        c16 = stg.tile([128, nb * ncol], f16, tag="c16_stage")
        nc.vector.tensor_copy(c16[:], stage[:])
        for b in range(nb):
            rt = r0 + b
            p = ps.tile([128, ncol], f16, tag="ps_tr")
            for ct in range(ncol // 128):
                nc.tensor.transpose(
                    p[:, ct * 128:(ct + 1) * 128],
                    c16[:, b * ncol + ct * 128:b * ncol + (ct + 1) * 128],
                    ident16[:])
            dst3 = dst[:].rearrange("p (b j) -> p b j", j=blk_stride)[
                :, 0:ncol // 128, rt * 128:(rt + 1) * 128]
            if ev[0] % 2 == 0:
                nc.vector.tensor_copy(dst3, p[:].rearrange(
                    "p (b j) -> p b j", j=128))
            else:
                nc.scalar.copy(dst3, p[:].rearrange("p (b j) -> p b j", j=128))
            ev[0] += 1


def _build_body(nc, tc, t):
    Qb, Kb, Vb = t["Qb"], t["Kb"], t["Vb"]
    Wq, Wk, Wv, Wo = t["Wq"], t["Wk"], t["Wv"], t["Wo"]
    bq, bk, bv, bo = t["bq"], t["bk"], t["bv"], t["bo"]
    Er, out, qer_dram = t["Er"], t["out"], t["qer_dram"]
    dbg = t.get("dbg")

    def dump(name, ap_f16):
        if dbg is not None and name in dbg:
            nc.gpsimd.dma_start(out=dbg[name].ap()[:, :], in_=ap_f16)

    from contextlib import ExitStack
    with ExitStack() as ctx:
        persist = ctx.enter_context(tc.tile_pool(name="persist", bufs=1))

        # ---- biases ----
        bq_col = persist.tile([128, NI], f32, tag="bq_col")
        bk_col = persist.tile([128, NI], f32, tag="bk_col")
        for jt in range(NI):
            nc.sync.dma_start(out=bq_col[:, jt:jt + 1],
                              in_=bq.ap()[jt * 128:(jt + 1) * 128].unsqueeze(1))
            nc.sync.dma_start(out=bk_col[:, jt:jt + 1],
                              in_=bk.ap()[jt * 128:(jt + 1) * 128].unsqueeze(1))
        bv_row = persist.tile([128, D], f32, tag="bv_row")
        nc.sync.dma_start(out=bv_row[:],
                          in_=AP(tensor=bv, offset=0, ap=[[0, 128], [1, D]]))
        bo_row = persist.tile([128, D], f32, tag="bo_row")
        nc.sync.dma_start(out=bo_row[:],
                          in_=AP(tensor=bo, offset=0, ap=[[0, 128], [1, D]]))

        # ---- persistent f16 operands ----
        qT = [persist.tile([128, S], f16, tag=f"qT{i}", name=f"qT{i}")
              for i in range(NI)]
        kT = [persist.tile([128, S], f16, tag=f"kT{i}", name=f"kT{i}")
              for i in range(NI)]
        v16 = [persist.tile([128, D], f16, tag=f"v16{i}", name=f"v16{i}")
               for i in range(NT)]
        woT = persist.tile([128, NI * D], f16, tag="woT")
        attn_outT = [persist.tile([128, S], f16, tag=f"aoT{i}", name=f"aoT{i}")
                     for i in range(NI)]
        erT2 = persist.tile([128, S], f16, tag="erT2")

        # ---- prologue: load + PE-transpose all weights/activations ----
        from concourse.masks import make_identity
        ident = persist.tile([128, 128], f32, tag="ident")
        make_identity(nc, ident[:])
        ident16 = persist.tile([128, 128], f16, tag="ident16")
        nc.vector.tensor_copy(ident16[:], ident[:])

        with tc.tile_pool(name="stage", bufs=2) as stg, \
             tc.tile_pool(name="ps_tr", bufs=2, space="PSUM") as ps_tr, \
             tc.tile_pool(name="ps_proj", bufs=4, space="PSUM") as psp:
            helpers = {"ident16": ident16, "ps": ps_tr, "ev": [0]}

            # Er first (small): pad cols to 128, transpose, dup halves
            with tc.tile_pool(name="er_stage", bufs=2) as erp:
                erT_tmp = erp.tile([128, S], f16, tag="erT_tmp", bufs=1)
                nate = erp.tile([128, NT * DK], f32, tag="er_nat", bufs=1)
                nc.sync.dma_start(
                    out=nate[:],
                    in_=AP(tensor=Er, offset=0,
                           ap=[[DK, 128], [128 * DK, NT], [1, DK]]))
                for et in range(NT):
                    pe = ps_tr.tile([128, D], f32, tag="ps_tr")
                    nc.tensor.transpose(pe[0:DK, 0:128],
                                        nate[:, et * DK:(et + 1) * DK],
                                        ident[:])
                    nc.vector.tensor_copy(
                        erT_tmp[0:DK, et * 128:(et + 1) * 128],
                        pe[0:DK, 0:128])
                nc.sync.dma_start(out=erT2[0:DK, :], in_=erT_tmp[0:DK, :])
                nc.sync.dma_start(out=erT2[DK:128, :], in_=erT_tmp[0:DK, :])

            pro_cm = tc.tile_pool(name="pro_qk", bufs=1)
            pro = pro_cm.__enter__()
            wqT = pro.tile([128, NI * D], f16, tag="wqT")
            wkT = pro.tile([128, NI * D], f16, tag="wkT")
            xqT = pro.tile([128, NI * S], f16, tag="xqT")
            xkT = pro.tile([128, NI * S], f16, tag="xkT")
            _pe_transpose_in(nc, stg, helpers, Wq, D, D, wqT, D)
            _pe_transpose_in(nc, stg, helpers, Qb, S, D, xqT, S)
            _pe_transpose_in(nc, stg, helpers, Wk, D, D, wkT, D)
            _pe_transpose_in(nc, stg, helpers, Kb, S, D, xkT, S)

            # q/k projections interleaved with phase-1 QEr for the matching
            # head pair (keeps the PE dense; qT[jt] is ready right before)
            srel_out = [pro.tile([128, RSW], f16, tag=f"srel_out{i}",
                                 name=f"srel_out{i}") for i in range(2)]
            for i in range(2):
                for si in range(NT):
                    g0 = OFFSP[si] + 128 * (si + 1)
                    nc.vector.memset(srel_out[i][:, g0:g0 + 128], NEG16)

            ev2 = [0]

            def qer_head(h):
                jb, jr = h // 2, 64 * (h % 2)
                so = srel_out[h % 2]
                reg = h * 128 * RSW
                for si in range(NT):
                    Wcw = 128 * (si + 1)
                    e0 = S - Wcw
                    for cs in range(0, Wcw, 512):
                        w = min(512, Wcw - cs)
                        p = psp.tile([128, 512], f32, tag="ps_proj")
                        nc.tensor.matmul(
                            p[:, :w],
                            qT[jb][jr:jr + 64, si * 128:(si + 1) * 128],
                            erT2[jr:jr + 64, e0 + cs:e0 + cs + w],
                            start=True, stop=True)
                        dstp = so[:, OFFSP[si] + cs:OFFSP[si] + cs + w]
                        if ev2[0] % 2 == 0:
                            nc.scalar.copy(dstp, p[:, :w])
                        else:
                            nc.vector.tensor_copy(dstp, p[:, :w])
                        ev2[0] += 1
                nc.sync.dma_start(
                    out=AP(tensor=qer_dram, offset=reg,
                           ap=[[RSW, 128], [1, RSW]]),
                    in_=so[:])

            for jt in range(NI):
                for wT, xT, bias_col, xT_out in ((wqT, xqT, bq_col, qT),
                                                 (wkT, xkT, bk_col, kT)):
                    for sh in range(2):
                        p = psp.tile([128, 512], f32, tag="ps_proj")
                        for ib in range(NI):
                            nc.tensor.matmul(
                                p[:],
                                wT[:, ib * D + jt * 128:ib * D + (jt + 1) * 128],
                                xT[:, ib * S + sh * 512:ib * S + (sh + 1) * 512],
                                start=(ib == 0), stop=(ib == NI - 1),
                            )
                        nc.vector.tensor_scalar_add(
                            xT_out[jt][:, sh * 512:(sh + 1) * 512], p[:],
                            bias_col[:, jt:jt + 1])
                qer_head(2 * jt)
                qer_head(2 * jt + 1)

            pro_cm.__exit__(None, None, None)
            pro2_cm = tc.tile_pool(name="pro_v", bufs=1)
            pro2 = pro2_cm.__enter__()
            wvT = pro2.tile([128, NI * D], f16, tag="wvT")
            xvT = pro2.tile([128, NI * S], f16, tag="xvT")
            _pe_transpose_in(nc, stg, helpers, Wv, D, D, wvT, D)
            _pe_transpose_in(nc, stg, helpers, Vb, S, D, xvT, S)

            # v projection: natural v16 [t-128, d] f16
            for tt in range(NT):
                for js, w in ((0, 512), (512, 256)):
                    p = psp.tile([128, 512], f32, tag="ps_proj")
                    for ib in range(NI):
                        nc.tensor.matmul(
                            p[:, :w],
                            xvT[:, ib * S + tt * 128:ib * S + (tt + 1) * 128],
                            wvT[:, ib * D + js:ib * D + js + w],
                            start=(ib == 0), stop=(ib == NI - 1),
                        )
                    nc.vector.tensor_add(v16[tt][:, js:js + w], p[:, :w],
                                         bv_row[:, js:js + w])

            _pe_transpose_in(nc, stg, helpers, Wo, D, D, woT, D)
            pro2_cm.__exit__(None, None, None)

        if dbg is not None:
            dump("d_qT0", qT[0][:])
            dump("d_kT0", kT[0][:])
            dump("d_v0", v16[0][:])
            dump("d_er", erT2[:])

        # ---- attention phase 2: skew-read -> scores -> softmax -> AV ----
        with tc.tile_pool(name="attn", bufs=1) as att, \
             tc.tile_pool(name="zpool", bufs=2) as zp, \
             tc.tile_pool(name="ps_qk", bufs=2, space="PSUM") as ps_qk, \
             tc.tile_pool(name="ps_tr", bufs=2, space="PSUM") as ps_tr2, \
             tc.tile_pool(name="ps_av", bufs=2, space="PSUM") as ps_av:
            srel16 = [att.tile([128, SREL_W], f16, tag=f"srel16_{i}",
                               name=f"srel16_{i}") for i in range(4)]
            A16 = [att.tile([128, SREL_W], f16, tag=f"A16_{i}",
                            name=f"A16_{i}") for i in range(2)]
            A16T = [att.tile([128, NT * S], f16, tag=f"A16T_{i}",
                             name=f"A16T_{i}") for i in range(4)]

            ones16 = att.tile([1, DK], f16, tag="ones16")
            nc.vector.memset(ones16[:], 1.0)
            rz8_all = {}
            aux_of = {}

            # 1/Z application pipeline, one stage per head so no queue ever
            # waits across engines:
            #   stage A (h=2j+1): rz8 cols -> PE transpose -> evict -> row
            #                     gathers (sync)
            #   stage B (h=2j+2): ones-broadcast matmuls -> rzb f16
            #   stage C (h=2j+3): AV on unnormalized A16T, multiply by rzb
            rzT = att.tile([NT, 256], f16, tag="rzT")
            rz1e = att.tile([1, S], f16, tag="rz1e")
            rz1o = att.tile([1, S], f16, tag="rz1o")
            rzb = att.tile([128, S], f16, tag="rzb")

            def rz_stage_a(j, rz_pair):
                aux = ps_av.tile([128, 512], f32, tag="avaux", bufs=1,
                                 name=f"aux{j}")
                nc.tensor.transpose(aux[0:32, 0:128], rz_pair[0][:],
                                    ident[:])
                nc.tensor.transpose(aux[0:32, 128:256], rz_pair[1][:],
                                    ident[:])
                nc.vector.tensor_copy(rzT[:], aux[0:NT, 0:256])
                nc.sync.dma_start(out=rz1e[:], in_=rzT[:, 0:128])
                nc.sync.dma_start(out=rz1o[:], in_=rzT[:, 128:256])
                return aux

            def rz_stage_b(j, aux):
                for c0 in (0, 512):
                    nc.tensor.matmul(aux[0:DK, 0:512], ones16[:],
                                     rz1e[:, c0:c0 + 512],
                                     start=True, stop=True)
                    nc.tensor.matmul(aux[DK:128, 0:512], ones16[:],
                                     rz1o[:, c0:c0 + 512],
                                     start=True, stop=True)
                    nc.vector.tensor_copy(rzb[:, c0:c0 + 512],
                                          aux[:, 0:512])

            def issue_av(j):
                pav = ps_av.tile([128, S], f32, tag="av", bufs=1,
                                 name=f"pav{j}")
                for idx in range(2):
                    aT = A16T[(2 * j + idx) % 4]
                    hh = 2 * j + idx
                    for ti in range(NT):
                        v16s = v16[ti][:, hh * DK:(hh + 1) * DK]
                        pieces = []
                        if ti < 4:
                            pieces.append((128 * ti, 512 - 128 * ti,
                                           ti == 0, ti == 3))
                        pieces.append((max(512, 128 * ti),
                                       1024 - max(512, 128 * ti),
                                       ti == 0, ti == NT - 1))
                        for (s0, w, st, sp) in pieces:
                            nc.tensor.matmul(
                                pav[64 * idx:64 * idx + 64, s0:s0 + w],
                                v16s,
                                aT[:, ti * S + s0:ti * S + s0 + w],
                                start=st, stop=sp)
                for c0 in (0, 512):
                    nc.vector.tensor_tensor(
                        out=attn_outT[j][:, c0:c0 + 512],
                        in0=pav[:, c0:c0 + 512], in1=rzb[:, c0:c0 + 512],
                        op=mybir.AluOpType.mult)

            for h in range(H):
                jb, jr = h // 2, 64 * (h % 2)
                par = h % 2
                s16, a16, a16T = srel16[h % 4], A16[par], A16T[h % 4]
                reg = h * 128 * RSW

                # skew reads (gpsimd SWDGE, idle engine): diagonal AP
                # realizes both the skew shift and the causal mask
                for si in range(NT):
                    Wcw = 128 * (si + 1)
                    nc.gpsimd.dma_start(
                        out=s16[:, OFFS[si]:OFFS[si] + Wcw],
                        in_=AP(tensor=qer_dram,
                               offset=reg + OFFSP[si] + 127,
                               ap=[[RSW - 1, 128], [1, Wcw]]))

                # scores per <=512 chunk: QK -> += srel (PE identity matmul)
                # -> exp with Z accum. A16T (unnormalized) is produced by PE
                # transposes emitted two strips behind the exps, evicted
                # from PSUM by vector/scalar with a strided AP.
                # chunk->zc col: strips 0-3 -> cols 0-3; strips 4-7 -> pairs
                zc = zp.tile([128, 12], f32, tag="zc", name=f"zc{h}")

                def emit_transposes(si):
                    if si < 4:
                        # small strips: XBAR transpose on the idle sync
                        # queue (single driver, no concurrency with PE path)
                        sl = a16[:, OFFS[si]:OFFS[si] + 128 * (si + 1)]
                        dst3 = a16T[:].rearrange("p (b j) -> p b j", j=S)[
                            :, 0:si + 1, si * 128:(si + 1) * 128]
                        nc.sync.dma_start_transpose(dst3, sl)
                        return
                    for tb0 in range(0, si + 1, 4):
                        nb = min(4, si + 1 - tb0)
                        ptr = ps_tr2.tile([128, 512], f16, tag="tr")
                        for k in range(nb):
                            tb = tb0 + k
                            nc.tensor.transpose(
                                ptr[:, k * 128:(k + 1) * 128],
                                a16[:, OFFS[si] + tb * 128:
                                    OFFS[si] + (tb + 1) * 128],
                                ident16[:])
                        dst3 = a16T[:].rearrange("p (b j) -> p b j", j=S)[
                            :, tb0:tb0 + nb, si * 128:(si + 1) * 128]
                        src3 = ptr[:, 0:nb * 128].rearrange(
                            "p (b j) -> p b j", j=128)
                        nc.vector.tensor_copy(dst3, src3)

                ci = 0
                for si in range(NT):
                    Wcw = 128 * (si + 1)
                    for cs in range(0, Wcw, 512):
                        w = min(512, Wcw - cs)
                        pqk = ps_qk.tile([128, 512], f32, tag="qk", bufs=3)
                        nc.tensor.matmul(
                            pqk[:, :w],
                            qT[jb][jr:jr + 64, si * 128:(si + 1) * 128],
                            kT[jb][jr:jr + 64, cs:cs + w],
                            start=True, stop=False)
                        nc.tensor.matmul(
                            pqk[:, :w], ident16[:],
                            s16[:, OFFS[si] + cs:OFFS[si] + cs + w],
                            start=False, stop=True)
                        nc.scalar.activation(
                            a16[:, OFFS[si] + cs:OFFS[si] + cs + w],
                            pqk[:, :w], mybir.ActivationFunctionType.Exp,
                            scale=0.125, accum_out=zc[:, ci:ci + 1])
                        ci += 1
                    if si >= 2:
                        emit_transposes(si - 2)
                emit_transposes(NT - 2)
                emit_transposes(NT - 1)

                if h == 0:
                    dump("d_s16", s16[:])

                zs = zp.tile([128, NT], f32, tag="zs", name=f"zs{h}")
                nc.vector.tensor_copy(zs[:, 0:4], zc[:, 0:4])
                zc2 = zc[:, 4:12].rearrange("p (a b) -> p a b", b=2)
                nc.vector.tensor_tensor(out=zs[:, 4:8], in0=zc2[:, :, 0],
                                        in1=zc2[:, :, 1],
                                        op=mybir.AluOpType.add)
                rz8 = zp.tile([128, 32], f32, tag="rz8", bufs=4,
                              name=f"rz8{h}")
                nc.vector.memset(rz8[:, NT:32], 1.0)
                nc.vector.reciprocal(rz8[:, 0:NT], zs[:])
                rz8_all[h] = rz8

                if h == 0:
                    dump("d_a16", a16[:])
                    dump("d_a16T", a16T[:])
                    if dbg is not None:
                        nc.sync.dma_start(out=dbg["d_z"].ap()[:, 0:NT],
                                          in_=zc[:])
                        nc.sync.dma_start(out=dbg["d_z"].ap()[:, NT:2 * NT],
                                          in_=rz8[:, 0:NT])

                # staged AV pipeline (see comment above); B and C both run
                # two heads after A so no cross-engine handoff ever waits
                if par == 1 and h >= 3:
                    jprev = (h - 3) // 2
                    rz_stage_b(jprev, aux_of.pop(jprev))
                    issue_av(jprev)
                    if h == 5 and dbg is not None:
                        dump("d_ao0", attn_outT[0][:])
                if par == 1:
                    aux_of[h // 2] = rz_stage_a(
                        h // 2, (rz8_all[h - 1], rz8_all[h]))
            jlast = H // 2 - 1
            rz_stage_b(jlast, aux_of.pop(jlast))
            issue_av(jlast)

        # ---- output projection ----
        # XBAR transposes emitted one jt behind the matmuls so the sync
        # queue never inline-waits; evicts alternate scalar/vector.
        with tc.tile_pool(name="ps_o", bufs=4, space="PSUM") as ps_o, \
             tc.tile_pool(name="stage_o", bufs=2) as stg:
            outs16 = [stg.tile([128, 4 * D], f16, tag=f"outs16_{sh}",
                               name=f"outs16_{sh}", bufs=1)
                      for sh in range(2)]
            pend = []
            for sh in range(2):
                for jt in range(NI):
                    p = ps_o.tile([128, 512], f32, tag="o")
                    for ib in range(NI):
                        nc.tensor.matmul(
                            p[:],
                            woT[:, ib * D + jt * 128:ib * D + (jt + 1) * 128],
                            attn_outT[ib][:, sh * 512:(sh + 1) * 512],
                            start=(ib == 0), stop=(ib == NI - 1))
                    o16 = stg.tile([128, 512], f16, tag="o16", bufs=3)
                    if jt % 2 == 0:
                        nc.scalar.copy(o16[:], p[:])
                    else:
                        nc.vector.tensor_copy(o16[:], p[:])
                    dst3 = outs16[sh][:].rearrange("p (b j) -> p b j", j=D)[
                        :, 0:4, jt * 128:(jt + 1) * 128]
                    pend.append((dst3, o16))
                    if len(pend) > 1:
                        d3, o = pend.pop(0)
                        nc.sync.dma_start_transpose(d3, o[:])
                for d3, o in pend:
                    nc.sync.dma_start_transpose(d3, o[:])
                pend = []
                # bias + writeback for this half (overlaps next half's
                # matmuls for sh=0)
                for st in range(4):
                    of = stg.tile([128, D], f32, tag="of", bufs=3)
                    nc.vector.tensor_add(of[:],
                                         outs16[sh][:, st * D:(st + 1) * D],
                                         bo_row[:])
                    row = (sh * 4 + st) * 128
                    nc.sync.dma_start(out=out.ap()[row:row + 128, :],
                                      in_=of[:])


_NC = None
_last_in_maps = None


def kernel(**inputs):
    global _NC, _last_in_maps
    if _NC is None:
        _NC = build_nc()
    Q = np.ascontiguousarray(np.asarray(inputs["Q"], dtype=np.float32))
    K = np.ascontiguousarray(np.asarray(inputs["K"], dtype=np.float32))
    V = np.ascontiguousarray(np.asarray(inputs["V"], dtype=np.float32))
    shared = {
        name: np.ascontiguousarray(np.asarray(inputs[name], dtype=np.float32))
        for name in ("Wq", "Wk", "Wv", "Wo", "bq", "bk", "bv", "bo", "Er")
    }
    in_maps = [
        {"Qb": Q[c], "Kb": K[c], "Vb": V[c], **shared} for c in range(N_CORES)
    ]
    _last_in_maps = in_maps
    res = run_bass_kernel_spmd(_NC, in_maps, list(range(N_CORES)))
    return np.stack([res.results[c]["out"] for c in range(N_CORES)], axis=0)


# revision 67
# speedup vs baseline: 1.2685x; 1.2685x over previous
"""Trainium2 Bass kernel for CustomMultiHeadAttention with relative position
bias (Music-Transformer skew), causal mask.

Sharding: pure data-parallel over batch - B=8 batches, one per NeuronCore.
Weights and Er are replicated. No collectives.

Per-core pipeline (all matmuls f16 inputs, f32 PSUM accumulate):
  - Prologue: load Q/K/V/W natural f32, cast f16, XBAR DMA-transpose into
    contraction-major layouts (no PE transposes anywhere).
  - Projections: qT/kT [dout, s] f16, v16 [t, d] f16 natural.
  - Per head: QEr strips -> f16 DRAM scratch laid out with -65504-filled
    128-col gaps between causal-packed strips; diagonal-AP HWDGE read
    realizes the skew AND the causal mask (gap values -> exp ~ 0).
    Scores computed natural [s-strip, t]: QK matmul -> PSUM, vector adds
    srel, scalar Exp (scale=1/8) with accum_out row-sums -> Z; reciprocal
    on [128,1] columns; gpsimd normalizes A in [s,t] layout; XBAR
    DMA-transpose A -> A16T [t, s]; AV accumulates both heads of a pair
    into one [128, S] PSUM tile (odd head at partition base 64).
  - Output projection f16 + XBAR transpose + bias -> DMA out.
"""

import numpy as np

import concourse.bass as bass
import concourse.tile as tile
from concourse import bacc, mybir
from concourse.bass import AP
from concourse.bass_utils import run_bass_kernel_spmd

N_CORES = 8
B, S, D, H, DK = 8, 1024, 768, 12, 64
NT = S // 128          # 8 s-tiles
NI = D // 128          # 6 d-blocks
f32 = mybir.dt.float32
f16 = mybir.dt.float16
NEG16 = -65504.0       # f16 lowest; exp(0.125*(x+NEG16)) == 0

# causal-packed column offsets (strip si width 128*(si+1))
OFFS = [0]
for _si in range(NT):
    OFFS.append(OFFS[-1] + 128 * (_si + 1))
SREL_W = OFFS[-1]                    # 4608
# gapped DRAM scratch offsets: strip si at OFFSP[si], then 128-col gap
OFFSP = [0]
for _si in range(NT):
    OFFSP.append(OFFSP[-1] + 128 * (_si + 1) + 128)
RSW = OFFSP[-1]                      # 5632
N_REG = 12                           # DRAM scratch regions (one per head)


def build_nc():
    nc = bacc.Bacc("TRN2", target_bir_lowering=False, debug=False,
                   num_devices=N_CORES)

    Qb = nc.dram_tensor("Qb", [S, D], f32, kind="ExternalInput")
    Kb = nc.dram_tensor("Kb", [S, D], f32, kind="ExternalInput")
    Vb = nc.dram_tensor("Vb", [S, D], f32, kind="ExternalInput")
    Wq = nc.dram_tensor("Wq", [D, D], f32, kind="ExternalInput")
    Wk = nc.dram_tensor("Wk", [D, D], f32, kind="ExternalInput")
    Wv = nc.dram_tensor("Wv", [D, D], f32, kind="ExternalInput")
    Wo = nc.dram_tensor("Wo", [D, D], f32, kind="ExternalInput")
    bq = nc.dram_tensor("bq", [D], f32, kind="ExternalInput")
    bk = nc.dram_tensor("bk", [D], f32, kind="ExternalInput")
    bv = nc.dram_tensor("bv", [D], f32, kind="ExternalInput")
    bo = nc.dram_tensor("bo", [D], f32, kind="ExternalInput")
    Er = nc.dram_tensor("Er", [S, DK], f32, kind="ExternalInput")
    out = nc.dram_tensor("out", [S, D], f32, kind="ExternalOutput")
    qer_dram = nc.dram_tensor("qer_scratch", [N_REG * 128 * RSW], f16)

    import os
    if os.environ.get("KDBG"):
        dbg = {
            "d_qT0": nc.dram_tensor("d_qT0", [128, S], f32, kind="ExternalOutput"),
            "d_kT0": nc.dram_tensor("d_kT0", [128, S], f32, kind="ExternalOutput"),
            "d_v0": nc.dram_tensor("d_v0", [128, D], f32, kind="ExternalOutput"),
            "d_er": nc.dram_tensor("d_er", [128, S], f32, kind="ExternalOutput"),
            "d_s16": nc.dram_tensor("d_s16", [128, SREL_W], f32, kind="ExternalOutput"),
            "d_a16": nc.dram_tensor("d_a16", [128, SREL_W], f32, kind="ExternalOutput"),
            "d_a16T": nc.dram_tensor("d_a16T", [128, NT * S], f32, kind="ExternalOutput"),
            "d_ao0": nc.dram_tensor("d_ao0", [128, S], f32, kind="ExternalOutput"),
            "d_z": nc.dram_tensor("d_z", [128, 16], f32, kind="ExternalOutput"),
        }
    else:
        dbg = None

    tensors = dict(Qb=Qb, Kb=Kb, Vb=Vb, Wq=Wq, Wk=Wk, Wv=Wv, Wo=Wo,
                   bq=bq, bk=bk, bv=bv, bo=bo, Er=Er, out=out,
                   qer_dram=qer_dram, dbg=dbg)
    with tile.TileContext(nc) as tc:
        _build_body(nc, tc, tensors)
    nc.compile()
    return nc


def _pe_transpose_in(nc, stg, helpers, src_dram, nrow, ncol, dst, blk_stride):
    """Load [nrow, ncol] f32 DRAM natural (one DMA per half-tensor),
    PE-transpose 128x128 blocks (f32), evict f32->f16 with a strided AP
    into dst [128, (ncol//128)*blk_stride]: src-col-block ct lands at
    dst cols [ct*blk_stride + rt*128, ...) for src-row-block rt."""
    nrb = nrow // 128
    nb = nrb // 2              # row-blocks per load (2 loads per tensor)
    ident16, ps = helpers["ident16"], helpers["ps"]
    ev = helpers["ev"]
    for ld in range(2):
        r0 = ld * nb
        stage = stg.tile([128, nb * ncol], f32, tag="nat_stage")
        src = AP(tensor=src_dram, offset=r0 * 128 * ncol,
                 ap=[[ncol, 128], [128 * ncol, nb], [1, ncol]])
        nc.sync.dma_start(out=stage[:], in_=src)
        c16 = stg.tile([128, nb * ncol], f16, tag="c16_stage")
        nc.vector.tensor_copy(c16[:], stage[:])
        for b in range(nb):
            rt = r0 + b
            p = ps.tile([128, ncol], f16, tag="ps_tr")
            for ct in range(ncol // 128):
                nc.tensor.transpose(
                    p[:, ct * 128:(ct + 1) * 128],
                    c16[:, b * ncol + ct * 128:b * ncol + (ct + 1) * 128],
                    ident16[:])
            dst3 = dst[:].rearrange("p (b j) -> p b j", j=blk_stride)[
                :, 0:ncol // 128, rt * 128:(rt + 1) * 128]
            if ev[0] % 2 == 0:
                nc.vector.tensor_copy(dst3, p[:].rearrange(
                    "p (b j) -> p b j", j=128))
            else:
                nc.scalar.copy(dst3, p[:].rearrange("p (b j) -> p b j", j=128))
            ev[0] += 1


def _build_body(nc, tc, t):
    Qb, Kb, Vb = t["Qb"], t["Kb"], t["Vb"]
    Wq, Wk, Wv, Wo = t["Wq"], t["Wk"], t["Wv"], t["Wo"]
    bq, bk, bv, bo = t["bq"], t["bk"], t["bv"], t["bo"]
    Er, out, qer_dram = t["Er"], t["out"], t["qer_dram"]
    dbg = t.get("dbg")

    def dump(name, ap_f16):
        if dbg is not None and name in dbg:
            nc.gpsimd.dma_start(out=dbg[name].ap()[:, :], in_=ap_f16)

    from contextlib import ExitStack
    with ExitStack() as ctx:
        persist = ctx.enter_context(tc.tile_pool(name="persist", bufs=1))

        # ---- biases ----
        bq_col = persist.tile([128, NI], f32, tag="bq_col")
        bk_col = persist.tile([128, NI], f32, tag="bk_col")
        for jt in range(NI):
            nc.sync.dma_start(out=bq_col[:, jt:jt + 1],
                              in_=bq.ap()[jt * 128:(jt + 1) * 128].unsqueeze(1))
            nc.sync.dma_start(out=bk_col[:, jt:jt + 1],
                              in_=bk.ap()[jt * 128:(jt + 1) * 128].unsqueeze(1))
        bv_row = persist.tile([128, D], f32, tag="bv_row")
        nc.sync.dma_start(out=bv_row[:],
                          in_=AP(tensor=bv, offset=0, ap=[[0, 128], [1, D]]))
        bo_row = persist.tile([128, D], f32, tag="bo_row")
        nc.sync.dma_start(out=bo_row[:],
                          in_=AP(tensor=bo, offset=0, ap=[[0, 128], [1, D]]))

        # ---- persistent f16 operands ----
        qT = [persist.tile([128, S], f16, tag=f"qT{i}", name=f"qT{i}")
              for i in range(NI)]
        kT = [persist.tile([128, S], f16, tag=f"kT{i}", name=f"kT{i}")
              for i in range(NI)]
        v16 = [persist.tile([128, D], f16, tag=f"v16{i}", name=f"v16{i}")
               for i in range(NT)]
        woT = persist.tile([128, NI * D], f16, tag="woT")
        attn_outT = [persist.tile([128, S], f16, tag=f"aoT{i}", name=f"aoT{i}")
                     for i in range(NI)]
        erT2 = persist.tile([128, S], f16, tag="erT2")

        # ---- prologue: load + PE-transpose all weights/activations ----
        from concourse.masks import make_identity
        ident = persist.tile([128, 128], f32, tag="ident")
        make_identity(nc, ident[:])
        ident16 = persist.tile([128, 128], f16, tag="ident16")
        nc.vector.tensor_copy(ident16[:], ident[:])

        with tc.tile_pool(name="stage", bufs=2) as stg, \
             tc.tile_pool(name="ps_tr", bufs=2, space="PSUM") as ps_tr, \
             tc.tile_pool(name="ps_proj", bufs=4, space="PSUM") as psp:
            helpers = {"ident16": ident16, "ps": ps_tr, "ev": [0]}

            # Er first (small): pad cols to 128, transpose, dup halves
            with tc.tile_pool(name="er_stage", bufs=2) as erp:
                erT_tmp = erp.tile([128, S], f16, tag="erT_tmp", bufs=1)
                nate = erp.tile([128, NT * DK], f32, tag="er_nat", bufs=1)
                nc.sync.dma_start(
                    out=nate[:],
                    in_=AP(tensor=Er, offset=0,
                           ap=[[DK, 128], [128 * DK, NT], [1, DK]]))
                for et in range(NT):
                    pe = ps_tr.tile([128, D], f32, tag="ps_tr")
                    nc.tensor.transpose(pe[0:DK, 0:128],
                                        nate[:, et * DK:(et + 1) * DK],
                                        ident[:])
                    nc.vector.tensor_copy(
                        erT_tmp[0:DK, et * 128:(et + 1) * 128],
                        pe[0:DK, 0:128])
                nc.sync.dma_start(out=erT2[0:DK, :], in_=erT_tmp[0:DK, :])
                nc.sync.dma_start(out=erT2[DK:128, :], in_=erT_tmp[0:DK, :])

            pro_cm = tc.tile_pool(name="pro_qk", bufs=1)
            pro = pro_cm.__enter__()
            wqT = pro.tile([128, NI * D], f16, tag="wqT")
            wkT = pro.tile([128, NI * D], f16, tag="wkT")
            xqT = pro.tile([128, NI * S], f16, tag="xqT")
            xkT = pro.tile([128, NI * S], f16, tag="xkT")
            _pe_transpose_in(nc, stg, helpers, Wq, D, D, wqT, D)
            _pe_transpose_in(nc, stg, helpers, Qb, S, D, xqT, S)
            _pe_transpose_in(nc, stg, helpers, Wk, D, D, wkT, D)
            _pe_transpose_in(nc, stg, helpers, Kb, S, D, xkT, S)

            # q/k projections interleaved with phase-1 QEr for the matching
            # head pair (keeps the PE dense; qT[jt] is ready right before)
            srel_out = [pro.tile([128, RSW], f16, tag=f"srel_out{i}",
                                 name=f"srel_out{i}") for i in range(2)]
            for i in range(2):
                for si in range(NT):
                    g0 = OFFSP[si] + 128 * (si + 1)
                    nc.vector.memset(srel_out[i][:, g0:g0 + 128], NEG16)

            ev2 = [0]

            def qer_head(h):
                jb, jr = h // 2, 64 * (h % 2)
                so = srel_out[h % 2]
                reg = h * 128 * RSW
                for si in range(NT):
                    Wcw = 128 * (si + 1)
                    e0 = S - Wcw
                    for cs in range(0, Wcw, 512):
                        w = min(512, Wcw - cs)
                        p = psp.tile([128, 512], f32, tag="ps_proj")
                        nc.tensor.matmul(
                            p[:, :w],
                            qT[jb][jr:jr + 64, si * 128:(si + 1) * 128],
                            erT2[jr:jr + 64, e0 + cs:e0 + cs + w],
                            start=True, stop=True)
                        dstp = so[:, OFFSP[si] + cs:OFFSP[si] + cs + w]
                        if ev2[0] % 2 == 0:
                            nc.scalar.copy(dstp, p[:, :w])
                        else:
                            nc.vector.tensor_copy(dstp, p[:, :w])
                        ev2[0] += 1
                nc.sync.dma_start(
                    out=AP(tensor=qer_dram, offset=reg,
                           ap=[[RSW, 128], [1, RSW]]),
                    in_=so[:])

            for jt in range(NI):
                for wT, xT, bias_col, xT_out in ((wqT, xqT, bq_col, qT),
                                                 (wkT, xkT, bk_col, kT)):
                    for sh in range(2):
                        p = psp.tile([128, 512], f32, tag="ps_proj")
                        for ib in range(NI):
                            nc.tensor.matmul(
                                p[:],
                                wT[:, ib * D + jt * 128:ib * D + (jt + 1) * 128],
                                xT[:, ib * S + sh * 512:ib * S + (sh + 1) * 512],
                                start=(ib == 0), stop=(ib == NI - 1),
                            )
                        nc.vector.tensor_scalar_add(
                            xT_out[jt][:, sh * 512:(sh + 1) * 512], p[:],
                            bias_col[:, jt:jt + 1])
                qer_head(2 * jt)
                qer_head(2 * jt + 1)

            pro_cm.__exit__(None, None, None)
            pro2_cm = tc.tile_pool(name="pro_v", bufs=1)
            pro2 = pro2_cm.__enter__()
            wvT = pro2.tile([128, NI * D], f16, tag="wvT")
            xvT = pro2.tile([128, NI * S], f16, tag="xvT")
            _pe_transpose_in(nc, stg, helpers, Wv, D, D, wvT, D)
            _pe_transpose_in(nc, stg, helpers, Vb, S, D, xvT, S)

            # v projection: natural v16 [t-128, d] f16
            for tt in range(NT):
                for js, w in ((0, 512), (512, 256)):
                    p = psp.tile([128, 512], f32, tag="ps_proj")
                    for ib in range(NI):
                        nc.tensor.matmul(
                            p[:, :w],
                            xvT[:, ib * S + tt * 128:ib * S + (tt + 1) * 128],
                            wvT[:, ib * D + js:ib * D + js + w],
                            start=(ib == 0), stop=(ib == NI - 1),
                        )
                    nc.vector.tensor_add(v16[tt][:, js:js + w], p[:, :w],
                                         bv_row[:, js:js + w])

            _pe_transpose_in(nc, stg, helpers, Wo, D, D, woT, D)
            pro2_cm.__exit__(None, None, None)

        if dbg is not None:
            dump("d_qT0", qT[0][:])
            dump("d_kT0", kT[0][:])
            dump("d_v0", v16[0][:])
            dump("d_er", erT2[:])

        # ---- attention phase 2: skew-read -> scores -> softmax -> AV ----
        with tc.tile_pool(name="attn", bufs=1) as att, \
             tc.tile_pool(name="zpool", bufs=2) as zp, \
             tc.tile_pool(name="ps_qk", bufs=2, space="PSUM") as ps_qk, \
             tc.tile_pool(name="ps_tr", bufs=2, space="PSUM") as ps_tr2, \
             tc.tile_pool(name="ps_av", bufs=2, space="PSUM") as ps_av:
            srel16 = [att.tile([128, SREL_W], f16, tag=f"srel16_{i}",
                               name=f"srel16_{i}") for i in range(4)]
            A16 = [att.tile([128, SREL_W], f16, tag=f"A16_{i}",
                            name=f"A16_{i}") for i in range(2)]
            A16T = [att.tile([128, NT * S], f16, tag=f"A16T_{i}",
                             name=f"A16T_{i}") for i in range(4)]

            ones16 = att.tile([1, DK], f16, tag="ones16")
            nc.vector.memset(ones16[:], 1.0)
            rz8_all = {}
            aux_of = {}

            # 1/Z application pipeline, one stage per head so no queue ever
            # waits across engines:
            #   stage A (h=2j+1): rz8 cols -> PE transpose -> evict -> row
            #                     gathers (sync)
            #   stage B (h=2j+2): ones-broadcast matmuls -> rzb f16
            #   stage C (h=2j+3): AV on unnormalized A16T, multiply by rzb
            rzT = att.tile([NT, 256], f16, tag="rzT")
            rz1e = att.tile([1, S], f16, tag="rz1e")
            rz1o = att.tile([1, S], f16, tag="rz1o")
            rzb = att.tile([128, S], f16, tag="rzb")

            def rz_stage_a(j, rz_pair):
                aux = ps_av.tile([128, 512], f32, tag="avaux", bufs=1,
                                 name=f"aux{j}")
                nc.tensor.transpose(aux[0:32, 0:128], rz_pair[0][:],
                                    ident[:])
                nc.tensor.transpose(aux[0:32, 128:256], rz_pair[1][:],
                                    ident[:])
                nc.vector.tensor_copy(rzT[:], aux[0:NT, 0:256])
                nc.sync.dma_start(out=rz1e[:], in_=rzT[:, 0:128])
                nc.sync.dma_start(out=rz1o[:], in_=rzT[:, 128:256])
                return aux

            def rz_stage_b(j, aux):
                for c0 in (0, 512):
                    nc.tensor.matmul(aux[0:DK, 0:512], ones16[:],
                                     rz1e[:, c0:c0 + 512],
                                     start=True, stop=True)
                    nc.tensor.matmul(aux[DK:128, 0:512], ones16[:],
                                     rz1o[:, c0:c0 + 512],
                                     start=True, stop=True)
                    nc.vector.tensor_copy(rzb[:, c0:c0 + 512],
                                          aux[:, 0:512])

            def issue_av(j):
                pav = ps_av.tile([128, S], f32, tag="av", bufs=1,
                                 name=f"pav{j}")
                for idx in range(2):
                    aT = A16T[(2 * j + idx) % 4]
                    hh = 2 * j + idx
                    for ti in range(NT):
                        v16s = v16[ti][:, hh * DK:(hh + 1) * DK]
                        pieces = []
                        if ti < 4:
                            pieces.append((128 * ti, 512 - 128 * ti,
                                           ti == 0, ti == 3))
                        pieces.append((max(512, 128 * ti),
                                       1024 - max(512, 128 * ti),
                                       ti == 0, ti == NT - 1))
                        for (s0, w, st, sp) in pieces:
                            nc.tensor.matmul(
                                pav[64 * idx:64 * idx + 64, s0:s0 + w],
                                v16s,
                                aT[:, ti * S + s0:ti * S + s0 + w],
                                start=st, stop=sp)
                for c0 in (0, 512):
                    nc.vector.tensor_tensor(
                        out=attn_outT[j][:, c0:c0 + 512],
                        in0=pav[:, c0:c0 + 512], in1=rzb[:, c0:c0 + 512],
                        op=mybir.AluOpType.mult)

            for h in range(H):
                jb, jr = h // 2, 64 * (h % 2)
                par = h % 2
                s16, a16, a16T = srel16[h % 4], A16[par], A16T[h % 4]
                reg = h * 128 * RSW

                # skew reads (gpsimd SWDGE, idle engine): diagonal AP
                # realizes both the skew shift and the causal mask
                for si in range(NT):
                    Wcw = 128 * (si + 1)
                    nc.gpsimd.dma_start(
                        out=s16[:, OFFS[si]:OFFS[si] + Wcw],
                        in_=AP(tensor=qer_dram,
                               offset=reg + OFFSP[si] + 127,
                               ap=[[RSW - 1, 128], [1, Wcw]]))

                # scores per <=512 chunk: QK -> += srel (PE identity matmul)
                # -> exp with Z accum. A16T (unnormalized) is produced by PE
                # transposes emitted two strips behind the exps, evicted
                # from PSUM by vector/scalar with a strided AP.
                # chunk->zc col: strips 0-3 -> cols 0-3; strips 4-7 -> pairs
                zc = zp.tile([128, 12], f32, tag="zc", name=f"zc{h}")

                def emit_transposes(si):
                    for tb0 in range(0, si + 1, 4):
                        nb = min(4, si + 1 - tb0)
                        ptr = ps_tr2.tile([128, 512], f16, tag="tr")
                        for k in range(nb):
                            tb = tb0 + k
                            nc.tensor.transpose(
                                ptr[:, k * 128:(k + 1) * 128],
                                a16[:, OFFS[si] + tb * 128:
                                    OFFS[si] + (tb + 1) * 128],
                                ident16[:])
                        dst3 = a16T[:].rearrange("p (b j) -> p b j", j=S)[
                            :, tb0:tb0 + nb, si * 128:(si + 1) * 128]
                        src3 = ptr[:, 0:nb * 128].rearrange(
                            "p (b j) -> p b j", j=128)
                        nc.vector.tensor_copy(dst3, src3)

                ci = 0
                for si in range(NT):
                    Wcw = 128 * (si + 1)
                    for cs in range(0, Wcw, 512):
                        w = min(512, Wcw - cs)
                        pqk = ps_qk.tile([128, 512], f32, tag="qk", bufs=3)
                        nc.tensor.matmul(
                            pqk[:, :w],
                            qT[jb][jr:jr + 64, si * 128:(si + 1) * 128],
                            kT[jb][jr:jr + 64, cs:cs + w],
                            start=True, stop=False)
                        nc.tensor.matmul(
                            pqk[:, :w], ident16[:],
                            s16[:, OFFS[si] + cs:OFFS[si] + cs + w],
                            start=False, stop=True)
                        nc.scalar.activation(
                            a16[:, OFFS[si] + cs:OFFS[si] + cs + w],
                            pqk[:, :w], mybir.ActivationFunctionType.Exp,
                            scale=0.125, accum_out=zc[:, ci:ci + 1])
                        ci += 1
                    if si >= 2:
                        emit_transposes(si - 2)
                emit_transposes(NT - 2)
                emit_transposes(NT - 1)

                if h == 0:
                    dump("d_s16", s16[:])

                zs = zp.tile([128, NT], f32, tag="zs", name=f"zs{h}")
                nc.vector.tensor_copy(zs[:, 0:4], zc[:, 0:4])
                zc2 = zc[:, 4:12].rearrange("p (a b) -> p a b", b=2)
                nc.vector.tensor_tensor(out=zs[:, 4:8], in0=zc2[:, :, 0],
                                        in1=zc2[:, :, 1],
                                        op=mybir.AluOpType.add)
                rz8 = zp.tile([128, 32], f32, tag="rz8", bufs=4,
                              name=f"rz8{h}")
                nc.vector.memset(rz8[:, NT:32], 1.0)
                nc.vector.reciprocal(rz8[:, 0:NT], zs[:])
                rz8_all[h] = rz8

                if h == 0:
                    dump("d_a16", a16[:])
                    dump("d_a16T", a16T[:])
                    if dbg is not None:
                        nc.sync.dma_start(out=dbg["d_z"].ap()[:, 0:NT],
                                          in_=zc[:])
                        nc.sync.dma_start(out=dbg["d_z"].ap()[:, NT:2 * NT],
                                          in_=rz8[:, 0:NT])

                # staged AV pipeline (see comment above); B and C both run
                # two heads after A so no cross-engine handoff ever waits
                if par == 1 and h >= 3:
                    jprev = (h - 3) // 2
                    rz_stage_b(jprev, aux_of.pop(jprev))
                    issue_av(jprev)
                    if h == 5 and dbg is not None:
                        dump("d_ao0", attn_outT[0][:])
                if par == 1:
                    aux_of[h // 2] = rz_stage_a(
                        h // 2, (rz8_all[h - 1], rz8_all[h]))
            jlast = H // 2 - 1
            rz_stage_b(jlast, aux_of.pop(jlast))
            issue_av(jlast)

        # ---- output projection ----
        # XBAR transposes emitted one jt behind the matmuls so the sync
        # queue never inline-waits; evicts alternate scalar/vector.
        with tc.tile_pool(name="ps_o", bufs=4, space="PSUM") as ps_o, \
             tc.tile_pool(name="stage_o", bufs=2) as stg:
            outs16 = [stg.tile([128, 4 * D], f16, tag=f"outs16_{sh}",
                               name=f"outs16_{sh}", bufs=1)
                      for sh in range(2)]
            pend = []
            for sh in range(2):
                for jt in range(NI):
                    p = ps_o.tile([128, 512], f32, tag="o")
                    for ib in range(NI):
                        nc.tensor.matmul(
                            p[:],
                            woT[:, ib * D + jt * 128:ib * D + (jt + 1) * 128],
                            attn_outT[ib][:, sh * 512:(sh + 1) * 512],
                            start=(ib == 0), stop=(ib == NI - 1))
                    o16 = stg.tile([128, 512], f16, tag="o16", bufs=3)
                    if jt % 2 == 0:
                        nc.scalar.copy(o16[:], p[:])
                    else:
                        nc.vector.tensor_copy(o16[:], p[:])
                    dst3 = outs16[sh][:].rearrange("p (b j) -> p b j", j=D)[
                        :, 0:4, jt * 128:(jt + 1) * 128]
                    pend.append((dst3, o16))
                    if len(pend) > 1:
                        d3, o = pend.pop(0)
                        nc.sync.dma_start_transpose(d3, o[:])
                for d3, o in pend:
                    nc.sync.dma_start_transpose(d3, o[:])
                pend = []
                # bias + writeback for this half (overlaps next half's
                # matmuls for sh=0)
                for st in range(4):
                    of = stg.tile([128, D], f32, tag="of", bufs=3)
                    nc.vector.tensor_add(of[:],
                                         outs16[sh][:, st * D:(st + 1) * D],
                                         bo_row[:])
                    row = (sh * 4 + st) * 128
                    nc.sync.dma_start(out=out.ap()[row:row + 128, :],
                                      in_=of[:])


_NC = None
_last_in_maps = None


def kernel(**inputs):
    global _NC, _last_in_maps
    if _NC is None:
        _NC = build_nc()
    Q = np.ascontiguousarray(np.asarray(inputs["Q"], dtype=np.float32))
    K = np.ascontiguousarray(np.asarray(inputs["K"], dtype=np.float32))
    V = np.ascontiguousarray(np.asarray(inputs["V"], dtype=np.float32))
    shared = {
        name: np.ascontiguousarray(np.asarray(inputs[name], dtype=np.float32))
        for name in ("Wq", "Wk", "Wv", "Wo", "bq", "bk", "bv", "bo", "Er")
    }
    in_maps = [
        {"Qb": Q[c], "Kb": K[c], "Vb": V[c], **shared} for c in range(N_CORES)
    ]
    _last_in_maps = in_maps
    res = run_bass_kernel_spmd(_NC, in_maps, list(range(N_CORES)))
    return np.stack([res.results[c]["out"] for c in range(N_CORES)], axis=0)
